# revision 1
# baseline (speedup 1.0000x reference)
"""Trainium2 Bass kernel for nn_LowRankRotatedSpaceIntervention.

Reference computation (B=8192, D=4096, r=512, k=128):
    sel  = subspaces[0]                  # shared index set (fast path)
    diff = (source - base) @ W           # [B, r]
    out  = base + diff[:, sel] @ W[:, sel].T

Only the selected k=128 columns of W matter:
    out = base + ((source - base) @ W_sel) @ W_sel.T,  W_sel = W[:, sel]

Sharding: data-parallel over batch across 8 NeuronCores; W_sel (2 MiB)
replicated. Host precomputes W_sel and W_sel.T (cheap) from subspaces[0].

Device kernel per core (batch shard 1024 rows, 8 blocks of 128):
    load base/source block [128, 4096] f32
    D  = source - base                    (DVE, output bf16)
    Dt = PE-transpose of D in [128,128] chunks (bf16, via identity matmul)
    T^T[k,128] = sum_j W_sel_chunk_j.T @ Dt_chunk_j   (32 bf16 matmuls, psum)
    out_block = base + (T^T).T @ W_selT   (8 fp32 matmuls N=512 + DVE add)
    store out_block

The correction term has rms ~0.25 vs base ~1.0, so bf16 rounding on the
first matmul contributes ~1e-3 absolute error on the output; the second
matmul and the final add are fp32.
"""

import os
import numpy as np
import ml_dtypes

import concourse.bass as bass
import concourse.tile as tile
from concourse import bacc, masks, mybir
from concourse.bass_utils import run_bass_kernel_spmd

N_CORES = 8
B_FULL = 8192
D = 4096
K = 128
BS = B_FULL // N_CORES  # 1024 rows per core
NB = BS // 128          # 8 blocks of 128 rows
NCH = D // 128          # 32 contraction chunks of 128

F32 = mybir.dt.float32
BF16 = mybir.dt.bfloat16


UNIT_LAYOUTS = {
    "pairs": [(0, 1), (2, 3), (4, 5), (6, 7)],
    "tail_singles": [(0, 1), (2, 3), (4, 5), (6,), (7,)],
    "singles": [(i,) for i in range(8)],
    # same unit shape, but tail singles transpose via DMA xbar instead of PE
    "tail_dma": [(0, 1), (2, 3), (4, 5), (6,), (7,)],
    "all_dma": [(0, 1), (2, 3), (4, 5), (6,), (7,)],
    # singles at both ends: fast pipeline ramp-up AND short tail chain
    "ends_singles": [(0,), (1,), (2, 3), (4, 5), (6,), (7,)],
}


def _build(mm1_dtype="bf16", mm2_f32r=False, layout="tail_singles", deep_bufs=False):
    nc = bacc.Bacc("TRN2", target_bir_lowering=False, debug=False)

    base_d = nc.dram_tensor("base", [BS, D], F32, kind="ExternalInput").ap()
    src_d = nc.dram_tensor("source", [BS, D], F32, kind="ExternalInput").ap()
    w1_dt = BF16 if mm1_dtype == "bf16" else F32
    # fp32r is bit-identical to fp32; declaring the whole w2/ttt path as
    # fp32r satisfies the BIR verifier's "rounded to FP32r" producer rule.
    w2_dt = mybir.dt.float32r if mm2_f32r else F32
    # w1: chunk-major W_sel: w1[p, 128*j + k] = W_sel[128*j + p, k]
    w1_d = nc.dram_tensor("w1", [128, D], w1_dt, kind="ExternalInput").ap()
    # w2: W_sel.T  (k on partitions)
    w2_d = nc.dram_tensor("w2", [K, D], w2_dt, kind="ExternalInput").ap()
    out_d = nc.dram_tensor("out", [BS, D], F32, kind="ExternalOutput").ap()

    with tile.TileContext(nc) as tc:
        with (
            tc.tile_pool(name="wpool", bufs=1) as wpool,
            tc.tile_pool(name="ipool", bufs=1) as ipool,
            tc.tile_pool(name="spool", bufs=4 if deep_bufs else 3) as spool,
            tc.tile_pool(name="dpool", bufs=2) as dpool,
            tc.tile_pool(name="dtpool", bufs=2) as dtpool,
            tc.tile_pool(name="ttpool", bufs=2) as ttpool,
            tc.tile_pool(name="opool", bufs=5 if deep_bufs else 4) as opool,
            tc.tile_pool(name="ptr", bufs=2, space="PSUM") as ptrpool,
            tc.tile_pool(name="pT", bufs=2, space="PSUM") as pTpool,
            tc.tile_pool(name="p2", bufs=4, space="PSUM") as p2pool,
        ):
            w1_sb = wpool.tile([128, D], w1_dt, tag="w1")
            nc.sync.dma_start(w1_sb[:], w1_d[:])
            w2_sb = wpool.tile([K, D], w2_dt, tag="w2")
            nc.sync.dma_start(w2_sb[:], w2_d[:])
            ident = ipool.tile([128, 128], w1_dt, tag="ident")
            masks.make_identity(nc, ident[:])

            # transposes per psum bank: bf16 bank holds 8 chunks, f32 bank 4
            per_bank = 8 if w1_dt == BF16 else 4
            bank_free = 128 * per_bank

            # blocks 0-5 in pairs (mm1 N=256); last two as singles so the
            # tail dependency chain (load->sub->transpose->mm1->mm2->store)
            # is short when the DMA stream runs dry of other work
            units = UNIT_LAYOUTS[layout]
            for unit in units:
                nu = len(unit)
                ots = []
                # Dt for the unit, block-major: dtt[p, D*par + 128*j + b]
                dtt = dtpool.tile([128, nu * D], w1_dt, tag="dtt")
                for par in range(nu):
                    i = unit[par]
                    # base loads straight into the output tile; the
                    # correction is accumulated in place later.
                    ot = opool.tile([128, D], F32, tag="ot")
                    nc.sync.dma_start(ot[:], base_d[128 * i : 128 * (i + 1), :])
                    st = spool.tile([128, D], F32, tag="st")
                    nc.sync.dma_start(st[:], src_d[128 * i : 128 * (i + 1), :])
                    ots.append(ot)

                    if mm1_dtype == "bf16":
                        db = dpool.tile([128, D], BF16, tag="db")
                        nc.vector.tensor_sub(db[:], st[:], ot[:])
                    else:
                        db = st  # subtract in place, keep f32
                        nc.vector.tensor_sub(db[:], st[:], ot[:])

                    use_dma_t = (layout == "all_dma") or (
                        layout == "tail_dma" and nu == 1
                    )
                    if use_dma_t:
                        # xbar transpose straight into dtt: with out viewed as
                        # [p, j, b], dtt[p, 128j+b] = db[b, 128j+p] — the same
                        # chunk layout the PE path produces.
                        d3 = dtt[:, D * par : D * (par + 1)].rearrange(
                            "p (j b) -> p j b", b=128
                        )
                        nc.sync.dma_start(d3, db[:], transpose=True)
                    else:
                        for g in range(NCH // per_bank):
                            ps = ptrpool.tile([128, bank_free], w1_dt, tag="ps")
                            for q in range(per_bank):
                                j = per_bank * g + q
                                nc.tensor.transpose(
                                    ps[:, 128 * q : 128 * (q + 1)],
                                    db[:, 128 * j : 128 * (j + 1)],
                                    ident[:],
                                )
                            nc.scalar.copy(
                                dtt[:, D * par + bank_free * g : D * par + bank_free * (g + 1)],
                                ps[:],
                            )

                # mm1: T^T for the unit, N=128*nu via 3D AP (par, b) over dtt
                dt3 = dtt[:].rearrange("p (par j b) -> p j par b", par=nu, b=128)
                pt = pTpool.tile([K, 128 * nu], F32, tag="pt")
                for j in range(NCH):
                    nc.tensor.matmul(
                        pt[:],
                        w1_sb[:, 128 * j : 128 * (j + 1)],
                        dt3[:, j],
                        start=(j == 0),
                        stop=(j == NCH - 1),
                    )
                ttt = ttpool.tile([K, 128 * nu], w2_dt, tag="ttt")
                nc.vector.tensor_copy(ttt[:], pt[:])

                for par in range(nu):
                    i = unit[par]
                    ot = ots[par]
                    for dj in range(D // 512):
                        p2t = p2pool.tile([128, 512], F32, tag="p2t")
                        lhs = ttt[:, 128 * par : 128 * (par + 1)]
                        rhs = w2_sb[:, 512 * dj : 512 * (dj + 1)]
                        nc.tensor.matmul(p2t[:], lhs, rhs, start=True, stop=True)
                        nc.vector.tensor_add(
                            ot[:, 512 * dj : 512 * (dj + 1)],
                            ot[:, 512 * dj : 512 * (dj + 1)],
                            p2t[:],
                        )
                    if nu == 1:
                        # stream the tail out in halves
                        half = D // 2
                        nc.sync.dma_start(
                            out_d[128 * i : 128 * (i + 1), :half], ot[:, :half]
                        )
                        nc.sync.dma_start(
                            out_d[128 * i : 128 * (i + 1), half:], ot[:, half:]
                        )
                    else:
                        nc.sync.dma_start(
                            out_d[128 * i : 128 * (i + 1), :], ot[:]
                        )

    nc.compile()
    return nc


_NC_CACHE = {}


def _get_nc(mm1_dtype, mm2_f32r, layout="tail_singles", deep_bufs=False):
    key = (mm1_dtype, mm2_f32r, layout, deep_bufs)
    if key not in _NC_CACHE:
        _NC_CACHE[key] = _build(mm1_dtype, mm2_f32r, layout, deep_bufs)
    return _NC_CACHE[key]


def make_in_maps(inputs, mm1_dtype="bf16"):
    base = np.ascontiguousarray(np.asarray(inputs["base"], dtype=np.float32))
    source = np.ascontiguousarray(np.asarray(inputs["source"], dtype=np.float32))
    subspaces = np.asarray(inputs["subspaces"])
    W = np.asarray(inputs["W"], dtype=np.float32)
    assert base.shape == (B_FULL, D) and source.shape == (B_FULL, D)

    sel = np.asarray(subspaces[0]).astype(np.int64)  # shared index set
    W_sel = np.ascontiguousarray(W[:, sel])          # [D, K] f32
    # chunk-major layout: w1[p, 128*j + k] = W_sel[128*j + p, k]
    w1 = np.ascontiguousarray(
        W_sel.reshape(NCH, 128, K).transpose(1, 0, 2).reshape(128, D)
    )
    if mm1_dtype == "bf16":
        w1 = w1.astype(ml_dtypes.bfloat16)
    w2 = np.ascontiguousarray(W_sel.T)               # [K, D] f32

    in_maps = []
    for c in range(N_CORES):
        in_maps.append(
            {
                "base": np.ascontiguousarray(base[c * BS : (c + 1) * BS]),
                "source": np.ascontiguousarray(source[c * BS : (c + 1) * BS]),
                "w1": w1,
                "w2": w2,
            }
        )
    return in_maps


def run(inputs, trace=False, mm1_dtype="bf16", mm2_f32r=False, layout="tail_singles", deep_bufs=False):
    nc = _get_nc(mm1_dtype, mm2_f32r, layout, deep_bufs)
    in_maps = make_in_maps(inputs, mm1_dtype)
    res = run_bass_kernel_spmd(nc, in_maps, list(range(N_CORES)), trace=trace)
    out = np.concatenate([r["out"] for r in res.results], axis=0)
    return out, res


def kernel(**inputs) -> np.ndarray:
    mm1_dtype = os.environ.get("LRI_MM1", "bf16")
    mm2_f32r = os.environ.get("LRI_MM2_F32R", "1") == "1"
    layout = os.environ.get("LRI_UNITS", "tail_singles")
    out, _ = run(inputs, trace=False, mm1_dtype=mm1_dtype, mm2_f32r=mm2_f32r, layout=layout)
    return out



# revision 2
# speedup vs baseline: 1.1256x; 1.1256x over previous
"""Trainium2 Bass kernel for nn_LowRankRotatedSpaceIntervention.

Reference computation (B=8192, D=4096, r=512, k=128):
    sel  = subspaces[0]                  # shared index set (fast path)
    diff = (source - base) @ W           # [B, r]
    out  = base + diff[:, sel] @ W[:, sel].T

Only the selected k=128 columns of W matter:
    out = base + ((source - base) @ W_sel) @ W_sel.T,  W_sel = W[:, sel]

Sharding: data-parallel over batch across 8 NeuronCores; W_sel (2 MiB)
replicated. Host precomputes W_sel and W_sel.T (cheap) from subspaces[0].

Device kernel per core (batch shard 1024 rows, 8 blocks of 128). The
kernel is HBM-bandwidth bound, so precision is cut wherever the 2e-2
harness tolerance allows:
  - both matmuls in bf16 (error on the small correction term only)
  - output stored as fp16 (halves store traffic; host upcasts to f32)
Per block:
    load base (f32, kept for the final add) and source (f32)
    D  = source - base   (bf16; columns split across DVE and GpSimd)
    Dt = PE-transpose of D in [128,128] chunks (bf16, identity matmul)
    T^T[k,128] = sum_j W_sel_chunk_j.T @ Dt_chunk_j   (bf16 matmuls, psum)
    out_block = fp16(base + (T^T).T @ W_selT)         (bf16 mm + DVE add)
    store out_block (fp16)
"""

import os
import numpy as np
import ml_dtypes

import concourse.bass as bass
import concourse.tile as tile
from concourse import bacc, masks, mybir
from concourse.bass_utils import run_bass_kernel_spmd

N_CORES = 8
B_FULL = 8192
D = 4096
K = 128
BS = B_FULL // N_CORES  # 1024 rows per core
NB = BS // 128          # 8 blocks of 128 rows
NCH = D // 128          # 32 contraction chunks of 128

F32 = mybir.dt.float32
BF16 = mybir.dt.bfloat16
F16 = mybir.dt.float16


UNIT_LAYOUTS = {
    "pairs": [(0, 1), (2, 3), (4, 5), (6, 7)],
    "tail_singles": [(0, 1), (2, 3), (4, 5), (6,), (7,)],
    "singles": [(i,) for i in range(8)],
    "ends_singles": [(0,), (1,), (2, 3), (4, 5), (6,), (7,)],
}


def _build(layout="tail_singles", sub_dve_cols=2048, bt_bufs=4, st_bufs=3,
           ot_bufs=3, out16=True):
    nc = bacc.Bacc("TRN2", target_bir_lowering=False, debug=False)

    base_d = nc.dram_tensor("base", [BS, D], F32, kind="ExternalInput").ap()
    src_d = nc.dram_tensor("source", [BS, D], F32, kind="ExternalInput").ap()
    # w1: chunk-major W_sel: w1[p, 128*j + k] = W_sel[128*j + p, k]
    w1_d = nc.dram_tensor("w1", [128, D], BF16, kind="ExternalInput").ap()
    # w2: W_sel.T  (k on partitions)
    w2_d = nc.dram_tensor("w2", [K, D], BF16, kind="ExternalInput").ap()
    out_dt = F16 if out16 else F32
    out_d = nc.dram_tensor("out", [BS, D], out_dt, kind="ExternalOutput").ap()

    with tile.TileContext(nc) as tc:
        with (
            tc.tile_pool(name="wpool", bufs=1) as wpool,
            tc.tile_pool(name="ipool", bufs=1) as ipool,
            tc.tile_pool(name="btpool", bufs=bt_bufs) as btpool,
            tc.tile_pool(name="stpool", bufs=st_bufs) as stpool,
            tc.tile_pool(name="dpool", bufs=2) as dpool,
            tc.tile_pool(name="dtpool", bufs=2) as dtpool,
            tc.tile_pool(name="ttpool", bufs=2) as ttpool,
            tc.tile_pool(name="opool", bufs=ot_bufs) as opool,
            tc.tile_pool(name="ptr", bufs=2, space="PSUM") as ptrpool,
            tc.tile_pool(name="pT", bufs=2, space="PSUM") as pTpool,
            tc.tile_pool(name="p2", bufs=4, space="PSUM") as p2pool,
        ):
            # gpsimd warmup: pay the custom-op IRAM load for tensor_sub at
            # t=0 (overlaps the initial DMA stream) instead of on block 0.
            if sub_dve_cols < D:
                warm = ipool.tile([128, 64], BF16, tag="warm")
                nc.gpsimd.memset(warm[:], 0.0)
                nc.gpsimd.tensor_sub(warm[:], warm[:], warm[:])

            w1_sb = wpool.tile([128, D], BF16, tag="w1")
            nc.sync.dma_start(w1_sb[:], w1_d[:])
            w2_sb = wpool.tile([K, D], BF16, tag="w2")
            nc.sync.dma_start(w2_sb[:], w2_d[:])
            ident = ipool.tile([128, 128], BF16, tag="ident")
            masks.make_identity(nc, ident[:])

            # bf16 transposes: one psum bank holds 8 [128,128] chunks
            per_bank = 8
            bank_free = 128 * per_bank

            units = UNIT_LAYOUTS[layout]
            for unit in units:
                nu = len(unit)
                bts = []
                # Dt for the unit, block-major: dtt[p, D*par + 128*j + b]
                dtt = dtpool.tile([128, nu * D], BF16, tag="dtt")
                for par in range(nu):
                    i = unit[par]
                    bt = btpool.tile([128, D], F32, tag="bt")
                    nc.sync.dma_start(bt[:], base_d[128 * i : 128 * (i + 1), :])
                    st = stpool.tile([128, D], F32, tag="st")
                    nc.sync.dma_start(st[:], src_d[128 * i : 128 * (i + 1), :])
                    bts.append(bt)

                    db = dpool.tile([128, D], BF16, tag="db")
                    sd = sub_dve_cols
                    if sd > 0:
                        nc.vector.tensor_sub(db[:, :sd], st[:, :sd], bt[:, :sd])
                    if sd < D:
                        nc.gpsimd.tensor_sub(db[:, sd:], st[:, sd:], bt[:, sd:])

                    for g in range(NCH // per_bank):
                        ps = ptrpool.tile([128, bank_free], BF16, tag="ps")
                        for q in range(per_bank):
                            j = per_bank * g + q
                            nc.tensor.transpose(
                                ps[:, 128 * q : 128 * (q + 1)],
                                db[:, 128 * j : 128 * (j + 1)],
                                ident[:],
                            )
                        nc.scalar.copy(
                            dtt[:, D * par + bank_free * g : D * par + bank_free * (g + 1)],
                            ps[:],
                        )

                # mm1: T^T for the unit, N=128*nu via 3D AP (par, b) over dtt
                dt3 = dtt[:].rearrange("p (par j b) -> p j par b", par=nu, b=128)
                pt = pTpool.tile([K, 128 * nu], F32, tag="pt")
                for j in range(NCH):
                    nc.tensor.matmul(
                        pt[:],
                        w1_sb[:, 128 * j : 128 * (j + 1)],
                        dt3[:, j],
                        start=(j == 0),
                        stop=(j == NCH - 1),
                    )
                ttt = ttpool.tile([K, 128 * nu], BF16, tag="ttt")
                nc.scalar.copy(ttt[:], pt[:])

                for par in range(nu):
                    i = unit[par]
                    bt = bts[par]
                    ot = opool.tile([128, D], out_dt, tag="ot")
                    for dj in range(D // 512):
                        p2t = p2pool.tile([128, 512], F32, tag="p2t")
                        lhs = ttt[:, 128 * par : 128 * (par + 1)]
                        rhs = w2_sb[:, 512 * dj : 512 * (dj + 1)]
                        nc.tensor.matmul(p2t[:], lhs, rhs, start=True, stop=True)
                        nc.vector.tensor_add(
                            ot[:, 512 * dj : 512 * (dj + 1)],
                            bt[:, 512 * dj : 512 * (dj + 1)],
                            p2t[:],
                        )
                        # stream the output out in halves as they complete
                        if dj == D // 1024 - 1:
                            nc.sync.dma_start(
                                out_d[128 * i : 128 * (i + 1), : D // 2],
                                ot[:, : D // 2],
                            )
                    nc.sync.dma_start(
                        out_d[128 * i : 128 * (i + 1), D // 2 :],
                        ot[:, D // 2 :],
                    )

    nc.compile()
    return nc


_NC_CACHE = {}


def _get_nc(**cfg):
    key = tuple(sorted(cfg.items()))
    if key not in _NC_CACHE:
        _NC_CACHE[key] = _build(**cfg)
    return _NC_CACHE[key]


def make_in_maps(inputs):
    base = np.ascontiguousarray(np.asarray(inputs["base"], dtype=np.float32))
    source = np.ascontiguousarray(np.asarray(inputs["source"], dtype=np.float32))
    subspaces = np.asarray(inputs["subspaces"])
    W = np.asarray(inputs["W"], dtype=np.float32)
    assert base.shape == (B_FULL, D) and source.shape == (B_FULL, D)

    sel = np.asarray(subspaces[0]).astype(np.int64)  # shared index set
    W_sel = np.ascontiguousarray(W[:, sel])          # [D, K] f32
    # chunk-major layout: w1[p, 128*j + k] = W_sel[128*j + p, k]
    w1 = np.ascontiguousarray(
        W_sel.reshape(NCH, 128, K).transpose(1, 0, 2).reshape(128, D)
    ).astype(ml_dtypes.bfloat16)
    w2 = np.ascontiguousarray(W_sel.T).astype(ml_dtypes.bfloat16)  # [K, D]

    in_maps = []
    for c in range(N_CORES):
        in_maps.append(
            {
                "base": np.ascontiguousarray(base[c * BS : (c + 1) * BS]),
                "source": np.ascontiguousarray(source[c * BS : (c + 1) * BS]),
                "w1": w1,
                "w2": w2,
            }
        )
    return in_maps


def run(inputs, trace=False, **cfg):
    nc = _get_nc(**cfg)
    in_maps = make_in_maps(inputs)
    res = run_bass_kernel_spmd(nc, in_maps, list(range(N_CORES)), trace=trace)
    out = np.concatenate(
        [np.asarray(r["out"], dtype=np.float32) for r in res.results], axis=0
    )
    return out, res


def _env_cfg():
    return dict(
        layout=os.environ.get("LRI_UNITS", "tail_singles"),
        sub_dve_cols=int(os.environ.get("LRI_SUB_DVE_COLS", "2048")),
        bt_bufs=int(os.environ.get("LRI_BT_BUFS", "4")),
        st_bufs=int(os.environ.get("LRI_ST_BUFS", "3")),
        ot_bufs=int(os.environ.get("LRI_OT_BUFS", "3")),
        out16=os.environ.get("LRI_OUT16", "1") == "1",
    )


def kernel(**inputs) -> np.ndarray:
    out, _ = run(inputs, trace=False, **_env_cfg())
    return out


# revision 3
# speedup vs baseline: 1.2962x; 1.1516x over previous
"""Trainium2 Bass kernel for nn_LowRankRotatedSpaceIntervention.

Reference computation (B=8192, D=4096, r=512, k=128):
    sel  = subspaces[0]                  # shared index set (fast path)
    diff = (source - base) @ W           # [B, r]
    out  = base + diff[:, sel] @ W[:, sel].T

Only the selected k=128 columns of W matter:
    out = base + ((source - base) @ W_sel) @ W_sel.T,  W_sel = W[:, sel]

Sharding: data-parallel over batch across 8 NeuronCores; W_sel (2 MiB)
replicated. Host precomputes W_sel and W_sel.T (cheap) from subspaces[0].

The kernel is HBM-bandwidth bound, so precision is cut wherever the
harness tolerance (2e-2) allows:
  - base/source land in SBUF as bf16 via SWDGE casting DMA (HBM reads
    stay f32; the cast frees SBUF so all 8 row-blocks stay resident and
    the load stream never stalls on buffer reuse)
  - both matmuls in bf16
  - output stored as fp16 (halves store traffic; host upcasts to f32)

Device kernel per core (batch shard 1024 rows, 8 blocks of 128):
    load bb=bf16(base), sb=bf16(source)     (gpsimd SWDGE cast DMA)
    D  = sb - bb                            (DVE, 16-bit 2x rate)
    Dt = PE-transpose of D in [128,128] chunks (identity matmul, psum)
    T^T[k,128] = sum_j W_sel_chunk_j.T @ Dt_chunk_j   (bf16 matmuls, psum)
    out_block = fp16(bb + (T^T).T @ W_selT)  (bf16 mm + DVE add)
    store out_block (fp16, in halves)
"""

import os
import numpy as np
import ml_dtypes

import concourse.bass as bass
import concourse.tile as tile
from concourse import bacc, masks, mybir
from concourse.bass_utils import run_bass_kernel_spmd

N_CORES = 8
B_FULL = 8192
D = 4096
K = 128
BS = B_FULL // N_CORES  # 1024 rows per core
NB = BS // 128          # 8 blocks of 128 rows
NCH = D // 128          # 32 contraction chunks of 128

F32 = mybir.dt.float32
BF16 = mybir.dt.bfloat16
F16 = mybir.dt.float16


UNIT_LAYOUTS = {
    "pairs": [(0, 1), (2, 3), (4, 5), (6, 7)],
    "tail_singles": [(0, 1), (2, 3), (4, 5), (6,), (7,)],
    "singles": [(i,) for i in range(8)],
    "ends_singles": [(0,), (1,), (2, 3), (4, 5), (6,), (7,)],
}


def _build(layout="tail_singles", sub_dve_cols=4096, bb_bufs=8, sb_bufs=4,
           db_bufs=3, ot_bufs=4):
    nc = bacc.Bacc("TRN2", target_bir_lowering=False, debug=False)

    base_d = nc.dram_tensor("base", [BS, D], F32, kind="ExternalInput").ap()
    src_d = nc.dram_tensor("source", [BS, D], F32, kind="ExternalInput").ap()
    # w1: chunk-major W_sel: w1[p, 128*j + k] = W_sel[128*j + p, k]
    w1_d = nc.dram_tensor("w1", [128, D], BF16, kind="ExternalInput").ap()
    # w2: W_sel.T  (k on partitions)
    w2_d = nc.dram_tensor("w2", [K, D], BF16, kind="ExternalInput").ap()
    out_d = nc.dram_tensor("out", [BS, D], F16, kind="ExternalOutput").ap()

    with tile.TileContext(nc) as tc:
        with (
            tc.tile_pool(name="wpool", bufs=1) as wpool,
            tc.tile_pool(name="ipool", bufs=1) as ipool,
            tc.tile_pool(name="bbpool", bufs=bb_bufs) as bbpool,
            tc.tile_pool(name="sbpool", bufs=sb_bufs) as sbpool,
            tc.tile_pool(name="dpool", bufs=db_bufs) as dpool,
            tc.tile_pool(name="dtpool", bufs=2) as dtpool,
            tc.tile_pool(name="ttpool", bufs=2) as ttpool,
            tc.tile_pool(name="opool", bufs=ot_bufs) as opool,
            tc.tile_pool(name="ptr", bufs=2, space="PSUM") as ptrpool,
            tc.tile_pool(name="pT", bufs=2, space="PSUM") as pTpool,
            tc.tile_pool(name="p2", bufs=4, space="PSUM") as p2pool,
        ):
            # gpsimd warmup: pay the custom-op IRAM load for tensor_sub at
            # t=0 (overlaps the initial DMA stream) instead of on block 0.
            if sub_dve_cols < D:
                warm = ipool.tile([128, 64], BF16, tag="warm")
                nc.gpsimd.memset(warm[:], 0.0)
                nc.gpsimd.tensor_sub(warm[:], warm[:], warm[:])

            w1_sb = wpool.tile([128, D], BF16, tag="w1")
            nc.sync.dma_start(w1_sb[:], w1_d[:])
            w2_sb = wpool.tile([K, D], BF16, tag="w2")
            nc.sync.dma_start(w2_sb[:], w2_d[:])
            ident = ipool.tile([128, 128], BF16, tag="ident")
            masks.make_identity(nc, ident[:])

            # bf16 transposes: one psum bank holds 8 [128,128] chunks
            per_bank = 8
            bank_free = 128 * per_bank

            units = UNIT_LAYOUTS[layout]
            for unit in units:
                nu = len(unit)
                bbs = []
                # Dt for the unit, block-major: dtt[p, D*par + 128*j + b]
                dtt = dtpool.tile([128, nu * D], BF16, tag="dtt")
                for par in range(nu):
                    i = unit[par]
                    bb = bbpool.tile([128, D], BF16, tag="bb")
                    nc.gpsimd.dma_start(bb[:], base_d[128 * i : 128 * (i + 1), :])
                    sb = sbpool.tile([128, D], BF16, tag="sb")
                    nc.gpsimd.dma_start(sb[:], src_d[128 * i : 128 * (i + 1), :])
                    bbs.append(bb)

                    db = dpool.tile([128, D], BF16, tag="db")
                    sd = sub_dve_cols
                    if sd > 0:
                        nc.vector.tensor_sub(db[:, :sd], sb[:, :sd], bb[:, :sd])
                    if sd < D:
                        nc.gpsimd.tensor_sub(db[:, sd:], sb[:, sd:], bb[:, sd:])

                    for g in range(NCH // per_bank):
                        ps = ptrpool.tile([128, bank_free], BF16, tag="ps")
                        for q in range(per_bank):
                            j = per_bank * g + q
                            nc.tensor.transpose(
                                ps[:, 128 * q : 128 * (q + 1)],
                                db[:, 128 * j : 128 * (j + 1)],
                                ident[:],
                            )
                        nc.scalar.copy(
                            dtt[:, D * par + bank_free * g : D * par + bank_free * (g + 1)],
                            ps[:],
                        )

                # mm1: T^T for the unit, N=128*nu via 3D AP (par, b) over dtt
                dt3 = dtt[:].rearrange("p (par j b) -> p j par b", par=nu, b=128)
                pt = pTpool.tile([K, 128 * nu], F32, tag="pt")
                for j in range(NCH):
                    nc.tensor.matmul(
                        pt[:],
                        w1_sb[:, 128 * j : 128 * (j + 1)],
                        dt3[:, j],
                        start=(j == 0),
                        stop=(j == NCH - 1),
                    )
                ttt = ttpool.tile([K, 128 * nu], BF16, tag="ttt")
                nc.scalar.copy(ttt[:], pt[:])

                for par in range(nu):
                    i = unit[par]
                    bb = bbs[par]
                    ot = opool.tile([128, D], F16, tag="ot")
                    for dj in range(D // 512):
                        p2t = p2pool.tile([128, 512], F32, tag="p2t")
                        lhs = ttt[:, 128 * par : 128 * (par + 1)]
                        rhs = w2_sb[:, 512 * dj : 512 * (dj + 1)]
                        nc.tensor.matmul(p2t[:], lhs, rhs, start=True, stop=True)
                        nc.vector.tensor_add(
                            ot[:, 512 * dj : 512 * (dj + 1)],
                            bb[:, 512 * dj : 512 * (dj + 1)],
                            p2t[:],
                        )
                        # stream the output out in halves as they complete
                        if dj == D // 1024 - 1:
                            nc.sync.dma_start(
                                out_d[128 * i : 128 * (i + 1), : D // 2],
                                ot[:, : D // 2],
                            )
                    nc.sync.dma_start(
                        out_d[128 * i : 128 * (i + 1), D // 2 :],
                        ot[:, D // 2 :],
                    )

    nc.compile()
    return nc


_NC_CACHE = {}


def _get_nc(**cfg):
    key = tuple(sorted(cfg.items()))
    if key not in _NC_CACHE:
        _NC_CACHE[key] = _build(**cfg)
    return _NC_CACHE[key]


def make_in_maps(inputs):
    base = np.ascontiguousarray(np.asarray(inputs["base"], dtype=np.float32))
    source = np.ascontiguousarray(np.asarray(inputs["source"], dtype=np.float32))
    subspaces = np.asarray(inputs["subspaces"])
    W = np.asarray(inputs["W"], dtype=np.float32)
    assert base.shape == (B_FULL, D) and source.shape == (B_FULL, D)

    sel = np.asarray(subspaces[0]).astype(np.int64)  # shared index set
    W_sel = np.ascontiguousarray(W[:, sel])          # [D, K] f32
    # chunk-major layout: w1[p, 128*j + k] = W_sel[128*j + p, k]
    w1 = np.ascontiguousarray(
        W_sel.reshape(NCH, 128, K).transpose(1, 0, 2).reshape(128, D)
    ).astype(ml_dtypes.bfloat16)
    w2 = np.ascontiguousarray(W_sel.T).astype(ml_dtypes.bfloat16)  # [K, D]

    in_maps = []
    for c in range(N_CORES):
        in_maps.append(
            {
                "base": np.ascontiguousarray(base[c * BS : (c + 1) * BS]),
                "source": np.ascontiguousarray(source[c * BS : (c + 1) * BS]),
                "w1": w1,
                "w2": w2,
            }
        )
    return in_maps


def run(inputs, trace=False, **cfg):
    nc = _get_nc(**cfg)
    in_maps = make_in_maps(inputs)
    res = run_bass_kernel_spmd(nc, in_maps, list(range(N_CORES)), trace=trace)
    out = np.concatenate(
        [np.asarray(r["out"], dtype=np.float32) for r in res.results], axis=0
    )
    return out, res


def _env_cfg():
    return dict(
        layout=os.environ.get("LRI_UNITS", "tail_singles"),
        sub_dve_cols=int(os.environ.get("LRI_SUB_DVE_COLS", "4096")),
        bb_bufs=int(os.environ.get("LRI_BB_BUFS", "8")),
        sb_bufs=int(os.environ.get("LRI_SB_BUFS", "4")),
        db_bufs=int(os.environ.get("LRI_DB_BUFS", "3")),
        ot_bufs=int(os.environ.get("LRI_OT_BUFS", "4")),
    )


def kernel(**inputs) -> np.ndarray:
    out, _ = run(inputs, trace=False, **_env_cfg())
    return out


# revision 4
# speedup vs baseline: 1.4016x; 1.0813x over previous
"""Trainium2 Bass kernel for nn_LowRankRotatedSpaceIntervention.

Reference computation (B=8192, D=4096, r=512, k=128):
    sel  = subspaces[0]                  # shared index set (fast path)
    diff = (source - base) @ W           # [B, r]
    out  = base + diff[:, sel] @ W[:, sel].T

Only the selected k=128 columns of W matter:
    out = base + ((source - base) @ W_sel) @ W_sel.T,  W_sel = W[:, sel]

Sharding: data-parallel over batch across 8 NeuronCores; W_sel (2 MiB)
replicated. Host precomputes W_sel and W_sel.T (cheap) from subspaces[0].

The kernel is HBM-bandwidth bound, so precision is cut wherever the
harness tolerance (2e-2) allows:
  - base/source land in SBUF as bf16 via SWDGE casting DMA (HBM reads
    stay f32; the cast frees SBUF so all 8 row-blocks stay resident and
    the load stream never stalls on buffer reuse)
  - both matmuls in bf16
  - output stored as fp16 (halves store traffic; host upcasts to f32)

Per core (batch shard 1024 rows, 8 blocks of 128 rows), per block i:
  front(i): load bb=bf16(base_i) whole + sb=bf16(source_i) in halves;
            per 1024-col group: sub (DVE), PE-transpose 8 chunks to
            psum, copy to SBUF (ACT); mm1 runs one group behind the
            copies; finally T^T -> ttt (bf16).
  back(i):  per 512-col chunk: mm2 (bf16) to psum, DVE add with bb to
            fp16 ot; store halves as they complete.
Blocks are software-pipelined with skew 1 — front(i+1) is issued ahead
of back(i) so the engine FIFOs never bury the next block's sub behind
the previous block's adds (kills the serial tail).
"""

import os
import numpy as np
import ml_dtypes

import concourse.bass as bass
import concourse.tile as tile
from concourse import bacc, masks, mybir
from concourse.bass_utils import run_bass_kernel_spmd

N_CORES = 8
B_FULL = 8192
D = 4096
K = 128
BS = B_FULL // N_CORES  # 1024 rows per core
NB = BS // 128          # 8 blocks of 128 rows
NCH = D // 128          # 32 contraction chunks of 128

F32 = mybir.dt.float32
BF16 = mybir.dt.bfloat16
F16 = mybir.dt.float16

PER_BANK = 8            # bf16 [128,128] transposes per psum bank
GCOLS = 128 * PER_BANK  # 1024 columns per transpose group
NG = D // GCOLS         # 4 groups per block


def _build(bb_bufs=8, sb_bufs=4, db_bufs=3, ot_bufs=4, sb_halves=True):
    nc = bacc.Bacc("TRN2", target_bir_lowering=False, debug=False)

    base_d = nc.dram_tensor("base", [BS, D], F32, kind="ExternalInput").ap()
    src_d = nc.dram_tensor("source", [BS, D], F32, kind="ExternalInput").ap()
    # w1: chunk-major W_sel: w1[p, 128*j + k] = W_sel[128*j + p, k]
    w1_d = nc.dram_tensor("w1", [128, D], BF16, kind="ExternalInput").ap()
    # w2: W_sel.T  (k on partitions)
    w2_d = nc.dram_tensor("w2", [K, D], BF16, kind="ExternalInput").ap()
    out_d = nc.dram_tensor("out", [BS, D], F16, kind="ExternalOutput").ap()

    with tile.TileContext(nc) as tc:
        with (
            tc.tile_pool(name="wpool", bufs=1) as wpool,
            tc.tile_pool(name="ipool", bufs=1) as ipool,
            tc.tile_pool(name="bbpool", bufs=bb_bufs) as bbpool,
            tc.tile_pool(name="sbpool", bufs=sb_bufs) as sbpool,
            tc.tile_pool(name="dpool", bufs=db_bufs) as dpool,
            tc.tile_pool(name="dtpool", bufs=3) as dtpool,
            tc.tile_pool(name="ttpool", bufs=2) as ttpool,
            tc.tile_pool(name="opool", bufs=ot_bufs) as opool,
            tc.tile_pool(name="ptr", bufs=2, space="PSUM") as ptrpool,
            tc.tile_pool(name="pT", bufs=2, space="PSUM") as pTpool,
            tc.tile_pool(name="p2", bufs=4, space="PSUM") as p2pool,
        ):
            w1_sb = wpool.tile([128, D], BF16, tag="w1")
            nc.sync.dma_start(w1_sb[:], w1_d[:])
            w2_sb = wpool.tile([K, D], BF16, tag="w2")
            nc.sync.dma_start(w2_sb[:], w2_d[:])
            ident = ipool.tile([128, 128], BF16, tag="ident")
            masks.make_identity(nc, ident[:])

            def front(i):
                rows = slice(128 * i, 128 * (i + 1))
                bb = bbpool.tile([128, D], BF16, tag="bb")
                nc.gpsimd.dma_start(bb[:], base_d[rows, :])
                sb = sbpool.tile([128, D], BF16, tag="sb")
                if sb_halves:
                    h = D // 2
                    nc.gpsimd.dma_start(sb[:, :h], src_d[rows, :h])
                    nc.gpsimd.dma_start(sb[:, h:], src_d[rows, h:])
                else:
                    nc.gpsimd.dma_start(sb[:], src_d[rows, :])

                db = dpool.tile([128, D], BF16, tag="db")
                dtt = dtpool.tile([128, D], BF16, tag="dtt")
                pt = pTpool.tile([K, 128], F32, tag="pt")

                def mm1_group(g):
                    for q in range(PER_BANK):
                        j = PER_BANK * g + q
                        nc.tensor.matmul(
                            pt[:],
                            w1_sb[:, 128 * j : 128 * (j + 1)],
                            dtt[:, 128 * j : 128 * (j + 1)],
                            start=(j == 0),
                            stop=(j == NCH - 1),
                        )

                for g in range(NG):
                    cols = slice(GCOLS * g, GCOLS * (g + 1))
                    nc.vector.tensor_sub(db[:, cols], sb[:, cols], bb[:, cols])
                    ps = ptrpool.tile([128, GCOLS], BF16, tag="ps")
                    for q in range(PER_BANK):
                        j = PER_BANK * g + q
                        nc.tensor.transpose(
                            ps[:, 128 * q : 128 * (q + 1)],
                            db[:, 128 * j : 128 * (j + 1)],
                            ident[:],
                        )
                    nc.scalar.copy(dtt[:, cols], ps[:])
                    # mm1 lags the copies by one group so the PE never
                    # stalls at the queue head waiting on the ACT copy
                    if g > 0:
                        mm1_group(g - 1)
                mm1_group(NG - 1)

                ttt = ttpool.tile([K, 128], BF16, tag="ttt")
                nc.scalar.copy(ttt[:], pt[:])
                return bb, ttt

            def back(i, bb, ttt):
                rows = slice(128 * i, 128 * (i + 1))
                ot = opool.tile([128, D], F16, tag="ot")
                for dj in range(D // 512):
                    p2t = p2pool.tile([128, 512], F32, tag="p2t")
                    nc.tensor.matmul(
                        p2t[:], ttt[:], w2_sb[:, 512 * dj : 512 * (dj + 1)],
                        start=True, stop=True,
                    )
                    nc.vector.tensor_add(
                        ot[:, 512 * dj : 512 * (dj + 1)],
                        bb[:, 512 * dj : 512 * (dj + 1)],
                        p2t[:],
                    )
                    if dj == D // 1024 - 1:
                        nc.sync.dma_start(out_d[rows, : D // 2], ot[:, : D // 2])
                nc.sync.dma_start(out_d[rows, D // 2 :], ot[:, D // 2 :])

            # skew-1 software pipeline: front(k+1) issues before back(k)
            state = front(0)
            for k in range(NB):
                nxt = front(k + 1) if k + 1 < NB else None
                back(k, *state)
                state = nxt

    nc.compile()
    return nc


_NC_CACHE = {}


def _get_nc(**cfg):
    key = tuple(sorted(cfg.items()))
    if key not in _NC_CACHE:
        _NC_CACHE[key] = _build(**cfg)
    return _NC_CACHE[key]


def make_in_maps(inputs):
    base = np.ascontiguousarray(np.asarray(inputs["base"], dtype=np.float32))
    source = np.ascontiguousarray(np.asarray(inputs["source"], dtype=np.float32))
    subspaces = np.asarray(inputs["subspaces"])
    W = np.asarray(inputs["W"], dtype=np.float32)
    assert base.shape == (B_FULL, D) and source.shape == (B_FULL, D)

    sel = np.asarray(subspaces[0]).astype(np.int64)  # shared index set
    W_sel = np.ascontiguousarray(W[:, sel])          # [D, K] f32
    # chunk-major layout: w1[p, 128*j + k] = W_sel[128*j + p, k]
    w1 = np.ascontiguousarray(
        W_sel.reshape(NCH, 128, K).transpose(1, 0, 2).reshape(128, D)
    ).astype(ml_dtypes.bfloat16)
    w2 = np.ascontiguousarray(W_sel.T).astype(ml_dtypes.bfloat16)  # [K, D]

    in_maps = []
    for c in range(N_CORES):
        in_maps.append(
            {
                "base": np.ascontiguousarray(base[c * BS : (c + 1) * BS]),
                "source": np.ascontiguousarray(source[c * BS : (c + 1) * BS]),
                "w1": w1,
                "w2": w2,
            }
        )
    return in_maps


def run(inputs, trace=False, **cfg):
    nc = _get_nc(**cfg)
    in_maps = make_in_maps(inputs)
    res = run_bass_kernel_spmd(nc, in_maps, list(range(N_CORES)), trace=trace)
    out = np.concatenate(
        [np.asarray(r["out"], dtype=np.float32) for r in res.results], axis=0
    )
    return out, res


def _env_cfg():
    return dict(
        bb_bufs=int(os.environ.get("LRI_BB_BUFS", "8")),
        sb_bufs=int(os.environ.get("LRI_SB_BUFS", "4")),
        db_bufs=int(os.environ.get("LRI_DB_BUFS", "3")),
        ot_bufs=int(os.environ.get("LRI_OT_BUFS", "4")),
        sb_halves=os.environ.get("LRI_SB_HALVES", "1") == "1",
    )


def kernel(**inputs) -> np.ndarray:
    out, _ = run(inputs, trace=False, **_env_cfg())
    return out
